# revision 49
# baseline (speedup 1.0000x reference)
"""Trainium2 Bass kernel for nn_BaseSelfAttention_88433376625006.

Computes: LayerNorm -> QKV projection -> 12-head causal self-attention
(seq 4096, dim 768) -> output projection, on 8 NeuronCores.

Sharding: 4 teams x 2 cores. Team t owns heads {3t, 3t+1, 3t+2}. Within a
team, core role 0 handles query rows {0..1023, 3072..4095} and role 1 rows
{1024..3071} (equal causal work). Each core computes LN + K/V for the keys
it needs, flash-style attention with the sim matrix in [k, q] layout, and a
partial output projection over its heads; the host scatters rows and sums
the 4 team partials. No collectives.

v2 design notes:
- All matmul inputs are bf16 (full PE rate, half SBUF, 2e-2 tolerance is
  plenty). PSUM stays f32; drains convert to bf16.
- No PE transposes: x rows are scaled by rstd (xs = x*rstd, bf16) and
  transposed by the DMA XBAR (SBUF->SBUF, 16x128 tiles) into [D, seq]
  layout. The remaining LayerNorm terms (-mu*rstd*colsum(W) + ln_b@W) are
  added by rank-1 correction matmuls: per 128-row block rb, the pair
  (nm_rb, ones) sits on partitions {32rb, 32rb+1} of a DMA-transposed
  stats tile, matching [t_cols; cb_cols] rows of a host-packed tile.
- Softmax skips max-subtraction (sim is O(1)); denominator rides the
  attention matmul as a ones-column of V.
- Output is written bf16 and upcast on host.
"""

import numpy as np
import ml_dtypes

BF16NP = ml_dtypes.bfloat16

HEADS = 12
N = 4096
D = 768
DH = 64
LN_EPS = 1e-5
TEAM_HEADS = 3
HD = TEAM_HEADS * DH  # head dims per core = 192

ROLE_SPEC = {
    0: dict(key_rows=4096, q0s=(0, 512, 3072, 3584)),
    1: dict(key_rows=3072, q0s=(1024, 1536, 2048, 2560)),
}

# Streaming schedule: chunks are processed in `order` (q-chunks early so Q is
# available); attention pairs for deferred q-tiles are emitted in windows as
# their key chunks complete, accumulating into SBUF partials; `fins` holds the
# straddle + leftover pairs + merge + normalize.  Entries: wins[qi] =
# [(position, pairs)...], fins[qi] = (position, leftover_pairs).
SCHED = {
    0: dict(
        order=[0, 1, 6, 5, 7, 2, 3, 4],
        wins={2: [(2, (0, 1, 2, 3)), (3, (10, 11)), (5, (4, 5)), (6, (6, 7))],
              3: [(4, (0, 1, 2, 3, 12, 13)), (5, (4, 5, 10, 11)), (6, (6, 7))]},
        swin={},
        fins={0: (0, ()), 1: (1, (0, 1)), 2: (7, (8, 9)), 3: (7, (8, 9))},
    ),
    1: dict(
        order=[0, 2, 3, 4, 5, 1],
        wins={0: [(1, (0, 1))], 1: [(2, (0, 1, 4, 5))],
              2: [(3, (0, 1, 4, 5, 6, 7))], 3: [(4, (0, 1, 4, 5, 6, 7, 8, 9))]},
        swin={},
        fins={0: (5, (2, 3)), 1: (5, (2, 3)), 2: (5, (2, 3)), 3: (5, (2, 3))},
    ),
}

_RUNNERS = None  # lazy build cache
STAGES = "ABC"  # debug: which stages to emit
XS_ON_ACT = False  # xs = x*rstd on ACT (Identity+scale) instead of DVE


# --------------------------------------------------------------------------
# neuronxcc workaround: this build rejects instructions with >1 sync wait.
# --------------------------------------------------------------------------
def _install_tile_patch():
    import concourse.tile as tile
    from concourse import mybir
    from concourse.vector_clock import ScopedClock

    if getattr(tile.TileContext, "_single_wait_patch", False):
        return

    def _patched_drain_and_barrier(self, tick_clock, wait_clock):
        nc = self.nc
        probe = nc.sync.nop(nofuse=True, hint="split_drain_waits")
        wait_clock.add_sem_waits(
            probe.ins, ScopedClock({None: tick_clock.global_clock})
        )
        si = probe.ins.sync_info
        waits = list(si.on_wait) if si and si.on_wait else []
        if len(waits) > 1:
            si.on_wait = waits[:1]
            for i in range(1, len(waits)):
                extra = nc.sync.nop(nofuse=True, hint=f"split_drain_waits_{i}")
                xsi = extra.ins.sync_info
                if xsi is None:
                    extra.ins.sync_info = mybir.SyncInfo(
                        on_wait=[waits[i]], on_update=[]
                    )
                else:
                    xsi.on_wait = [waits[i]]
        nc.sync.drain()
        nc.all_engine_barrier()
        popped = nc._tile_sem_poison_stack.pop()
        assert popped is self._sem_poison
        nc.clear_and_free_semaphores(list(self.sems.allocated().values()))
        nc.all_engine_barrier()

    tile.TileContext._drain_and_barrier = _patched_drain_and_barrier

    _orig_commit = tile.TileContext._commit_instruction

    def _patched_commit_instruction(self, inst, lazy_reg_writes=True):
        si = getattr(inst, "sync_info", None)
        if (
            si is not None
            and si.on_wait
            and len(si.on_wait) > 1
            and inst.engine != mybir.EngineType.Unassigned
        ):
            waits = list(si.on_wait)
            si.on_wait = waits[-1:]
            for w in waits[:-1]:
                nop = mybir.InstNoOp(
                    name=self.nc.get_next_instruction_name(),
                    sync_info=mybir.SyncInfo(on_wait=[w], on_update=[]),
                    bass_nofuse=True,
                    engine=inst.engine,
                )
                _orig_commit(self, nop, lazy_reg_writes=False)
        return _orig_commit(self, inst, lazy_reg_writes=lazy_reg_writes)

    tile.TileContext._commit_instruction = _patched_commit_instruction
    tile.TileContext._single_wait_patch = True


# --------------------------------------------------------------------------
# Per-device program dispatch (different programs on different cores).
# --------------------------------------------------------------------------
def _make_runner(nc):
    import jax
    from concourse import mybir
    from concourse.bass2jax import _bass_exec_p, install_neuronx_cc_hook

    install_neuronx_cc_hook()
    pid_name = nc.partition_id_tensor.name if nc.partition_id_tensor else None
    in_names, out_names, out_avals, zero_outs = [], [], [], []
    for alloc in nc.m.functions[0].allocations:
        if not isinstance(alloc, mybir.MemoryLocationSet):
            continue
        name = alloc.memorylocations[0].name
        if alloc.kind == "ExternalInput":
            if name != pid_name:
                in_names.append(name)
        elif alloc.kind == "ExternalOutput":
            shape = tuple(alloc.tensor_shape)
            dtype = mybir.dt.np(alloc.dtype)
            out_names.append(name)
            out_avals.append(jax.core.ShapedArray(shape, dtype))
            zero_outs.append(np.zeros(shape, dtype))
    n_params = len(in_names)
    all_names = in_names + out_names + ([pid_name] if pid_name else [])
    donate = tuple(range(n_params, n_params + len(out_names)))

    def _body(*args):
        return tuple(
            _bass_exec_p.bind(
                *args,
                out_avals=tuple(out_avals),
                in_names=tuple(all_names),
                out_names=tuple(out_names),
                lowering_input_output_aliases=(),
                sim_require_finite=True,
                sim_require_nnan=True,
                nc=nc,
            )
        )

    jitted = jax.jit(_body, donate_argnums=donate, keep_unused=True)
    jitted_nodonate = jax.jit(_body, keep_unused=True)

    def run(in_map, device, core_id=0):
        args = [jax.device_put(np.asarray(in_map[n]), device) for n in in_names]
        args += [jax.device_put(z.copy(), device) for z in zero_outs]
        if pid_name is not None:
            args.append(jax.device_put(np.array([[core_id]], np.uint32), device))
        outs = jitted(*args)
        return {n: outs[i] for i, n in enumerate(out_names)}

    def stage(in_map, device, core_id=0):
        args = [jax.device_put(np.asarray(in_map[n]), device) for n in in_names]
        args += [jax.device_put(z, device) for z in zero_outs]
        if pid_name is not None:
            args.append(jax.device_put(np.array([[core_id]], np.uint32), device))
        return args

    def run_staged(args):
        return jitted_nodonate(*args)

    run.stage = stage
    run.run_staged = run_staged
    run.out_names = out_names
    return run


# --------------------------------------------------------------------------
# The kernel program for one role.
# --------------------------------------------------------------------------
def _build_role_program(role, masked=False):
    import concourse.bass as bass
    import concourse.tile as tile
    from concourse import mybir

    F32 = mybir.dt.float32
    BF = mybir.dt.bfloat16
    AF = mybir.ActivationFunctionType
    ALU = mybir.AluOpType

    spec = ROLE_SPEC[role]
    KR = spec["key_rows"]  # key rows this core needs
    q0s = spec["q0s"]  # global start row of each 512-row query tile
    KC = KR // 512  # number of 512-row chunks
    KB = KR // 128  # number of 128-row key blocks
    q_chunks = {q0 // 512: qi for qi, q0 in enumerate(q0s)}  # chunk -> q index
    sched = SCHED[role]

    nc = bass.Bass(enable_partition_id=False)

    x_in = nc.declare_dram_parameter("x", [KR, D], F32, isOutput=False)
    wqk_in = nc.declare_dram_parameter("wqk", [128, 6, 384], BF, isOutput=False)
    wv_in = nc.declare_dram_parameter("wv", [128, 6, 192], BF, isOutput=False)
    cqk_in = nc.declare_dram_parameter("corrqk", [2, 384], BF, isOutput=False)
    cv_in = nc.declare_dram_parameter("corrv", [2, 192], BF, isOutput=False)
    wo_in = nc.declare_dram_parameter("wo", [128, 1536], BF, isOutput=False)
    mk_in = nc.declare_dram_parameter("maskv", [128, KB], F32, isOutput=False)
    mb_in = nc.declare_dram_parameter("mb", [128, 128], BF, isOutput=False)
    on_in = nc.declare_dram_parameter("ones", [1, 512], BF, isOutput=False)
    y_out = nc.declare_dram_parameter("out", [2048, D], BF, isOutput=True)

    with tile.TileContext(nc) as tc:
        with (
            tc.tile_pool(name="persist", bufs=1) as pp,
            tc.tile_pool(name="work", bufs=3) as wk,
            tc.tile_pool(name="xstp", bufs=4) as xsp,
            tc.tile_pool(name="xntp", bufs=3) as xp,
            tc.tile_pool(name="xtp", bufs=6) as xtp,
            tc.tile_pool(name="small", bufs=6) as sm,
            tc.tile_pool(name="expp", bufs=6) as ep,
            tc.tile_pool(name="psga", bufs=2, space="PSUM") as ps_a,
            tc.tile_pool(name="psim", bufs=2, space="PSUM") as ps_b,
            tc.tile_pool(name="pso", bufs=1, space="PSUM") as ps_o,
        ):
            # ---- persistent tiles ----
            ones_row = pp.tile([1, 512], BF, tag="ones_row")
            nc.sync.dma_start(out=ones_row, in_=on_in[:])
            maskv = pp.tile([128, KB], F32, tag="maskv")
            nc.sync.dma_start(out=maskv, in_=mk_in[:])
            mb = pp.tile([128, 128], BF, tag="mb")
            nc.sync.dma_start(out=mb, in_=mb_in[:])
            eps_t = pp.tile([128, 1], F32, tag="eps")
            nc.vector.memset(eps_t, LN_EPS)
            wqk = pp.tile([128, 6, 384], BF, tag="wqk")
            nc.gpsimd.dma_start(out=wqk, in_=wqk_in[:])
            wv = pp.tile([128, 6, 192], BF, tag="wv")
            nc.gpsimd.dma_start(out=wv, in_=wv_in[:])
            corrqk = pp.tile([2, 384], BF, tag="corrqk")
            nc.gpsimd.dma_start(out=corrqk, in_=cqk_in[:])
            corrv = pp.tile([2, 192], BF, tag="corrv")
            nc.gpsimd.dma_start(out=corrv, in_=cv_in[:])
            wo = pp.tile([128, 1536], BF, tag="wo")
            nc.gpsimd.dma_start(out=wo, in_=wo_in[:])

            # per-chunk / per-qtile persistent tiles => fine-grained deps
            # QA[qi]: qh0 on p0-63, qh1 on p64-127.  QB2[qi]: qh2 on p64-127.
            # KA[c]: kh0 p0-63, kh1 p64-127.  T2[c]: kh2 on p64-127.
            QA = [pp.tile([128, 512], BF, name=f"QA{qi}", tag=f"QA{qi}") for qi in range(4)]
            QB2 = [pp.tile([128, 512], BF, name=f"QB2_{qi}", tag=f"QB2_{qi}") for qi in range(4)]
            KA = [pp.tile([128, 512], BF, name=f"KA{c}", tag=f"KA{c}") for c in range(KC)]
            T2 = [pp.tile([128, 512], BF, name=f"T2_{c}", tag=f"T2_{c}") for c in range(KC)]
            vv = [
                pp.tile([128, 4, 3, 66], BF, name=f"vv{c}", tag=f"vv{c}")
                for c in range(KC)
            ]
            oq = [pp.tile([128, 512], BF, name=f"oq{qi}", tag=f"oq{qi}") for qi in range(4)]
            oq2 = [pp.tile([64, 512], BF, name=f"oq2_{qi}", tag=f"oq2_{qi}") for qi in range(4)]
            # SBUF partial accumulators for windowed attention (f32)
            pb = {
                qi: [
                    pp.tile([65, 512], F32, name=f"pb{qi}_{h}", tag=f"pb{qi}_{h}")
                    for h in range(3)
                ]
                for qi in sched["wins"]
            }

            # psum->sbuf drains, round-robin with a per-stage ACT share.
            # set_cp(k>0): 1/k of copies on ACT; set_cp(k<0): 1/|k| on DVE.
            _cp_state = [0, 2]

            def cp(out, in_):
                _cp_state[0] += 1
                k = _cp_state[1]
                on_act = (
                    _cp_state[0] % k == 0 if k > 0 else _cp_state[0] % (-k) != 0
                )
                with nc.allow_low_precision(reason="psum drain rounds to bf16"):
                    if on_act:
                        nc.scalar.copy(out=out, in_=in_)
                    else:
                        nc.vector.tensor_copy(out=out, in_=in_)

            def set_cp(act_every):
                _cp_state[1] = act_every

            # ---------- stage A: LN + DMA-transpose + QKV for one chunk ----
            # Emitted as a generator of PE-sized steps so the emission driver
            # can interleave attention pairs (ACT-heavy) with QKV groups
            # (PE-heavy) in the in-order engine queues.
            def stage_a_chunk(c):
                set_cp(4)  # stage-A drains mostly on DVE (ACT busy with exp)
                qi = q_chunks.get(c)
                mvs = sm.tile([128, 4, 2], F32, tag="mvs", name=f"mvs{c}")
                x_ts = []
                for rb in range(4):
                    row0 = c * 512 + rb * 128
                    x_t = xtp.tile([128, D], F32, tag="x_t", name=f"x{c}_{rb}")
                    x_ts.append(x_t)
                    nc.sync.dma_start(out=x_t, in_=x_in[row0 : row0 + 128, :])
                    st = sm.tile([128, 2, 6], F32, tag="st", name=f"st{c}_{rb}")
                    nc.vector.bn_stats(out=st[:, 0, :], in_=x_t[:, 0:512])
                    nc.vector.bn_stats(out=st[:, 1, :], in_=x_t[:, 512:768])
                    nc.vector.bn_aggr(out=mvs[:, rb, :], in_=st)
                # rstd = exp(-0.5*ln(var+eps)): keeps ACT inside the exp/ln/copy
                # activation table (Sqrt would force a 1.3us table reload per use)
                lns = sm.tile([128, 4], F32, tag="lns", name=f"lns{c}")
                rstds = sm.tile([128, 4], F32, tag="rstds", name=f"rss{c}")
                if c == 0:  # latency-critical first chunk: per-rowblock chain
                    for rb in range(4):
                        nc.scalar.activation(
                            out=lns[:, rb : rb + 1], in_=mvs[:, rb, 1:2],
                            func=AF.Ln, bias=eps_t, scale=1.0,
                        )
                        nc.scalar.activation(
                            out=rstds[:, rb : rb + 1], in_=lns[:, rb : rb + 1],
                            func=AF.Exp, bias=0.0, scale=-0.5,
                        )
                else:
                    nc.scalar.activation(
                        out=lns, in_=mvs[:, :, 1], func=AF.Ln, bias=eps_t, scale=1.0
                    )
                    nc.scalar.activation(
                        out=rstds, in_=lns, func=AF.Exp, bias=0.0, scale=-0.5
                    )

                # nm tile: col 128*rb = -mu*rstd of rowblock rb, other cols = 1.
                # DMA-transposed into nmt [128, 4, 128]: partition 0 then holds
                # the full-width row nm_n over queries n=(rb*128+j); partition 1
                # holds ones.
                nmb = sm.tile([128, 512], BF, tag="nmb", name=f"nmb{c}")
                nc.gpsimd.memset(nmb, 1.0)
                negm = sm.tile([128, 4], F32, tag="negm", name=f"negm{c}")
                nc.gpsimd.tensor_scalar(
                    out=negm, in0=mvs[:, :, 0], scalar1=-1.0, scalar2=None,
                    op0=ALU.mult,
                )
                with nc.allow_low_precision(reason="LN correction rounds to bf16"):
                    nc.gpsimd.tensor_tensor(
                        out=nmb[:, 0:512:128], in0=negm, in1=rstds, op=ALU.mult
                    )
                nmt = sm.tile([128, 4, 128], BF, tag="nmt", name=f"nmt{c}")
                nc.sync.dma_start_transpose(out=nmt, in_=nmb)

                # xs = x*rstd (bf16), DMA-transposed into [D, seq] layout
                xsT = xp.tile([128, 4, 6, 128], BF, tag="xsT", name=f"xsT{c}")
                for rb in range(4):
                    xs = xsp.tile([128, D], BF, tag="xs", name=f"xs{c}_{rb}")
                    with nc.allow_low_precision(reason="xs rounds to bf16"):
                        if XS_ON_ACT:
                            nc.scalar.activation(
                                out=xs, in_=x_ts[rb], func=AF.Identity,
                                bias=0.0, scale=rstds[:, rb : rb + 1],
                            )
                        else:
                            # on Pool: DVE is loaded with bn_stats/drains, Pool idles
                            nc.gpsimd.tensor_scalar(
                                out=xs, in0=x_ts[rb], scalar1=rstds[:, rb : rb + 1],
                                scalar2=None, op0=ALU.mult,
                            )
                    nc.sync.dma_start_transpose(out=xsT[:, rb, :, :], in_=xs)
                yield  # front matter emitted; PE steps follow

                # Q/K groups: cols 0-127=[qh0|qh1], 128-255=[kh0|kh1],
                # 256-383=[qh2|kh2]
                groups = ([(0, "qa")] if qi is not None else []) + [
                    (128, "ka"),
                    (256, "t2"),
                ]
                for g0, kind in groups:
                    gp = ps_a.tile([128, 512], F32, tag="mma", name=f"gp{c}_{g0}")
                    for d in range(6):
                        nc.tensor.matmul(
                            gp, wqk[:, d, g0 : g0 + 128], xsT[:, :, d, :],
                            start=(d == 0), stop=False,
                        )
                    nc.tensor.matmul(
                        gp, corrqk[0:2, g0 : g0 + 128], nmt[0:2, :, :],
                        start=False, stop=True,
                    )
                    set_cp(4)
                    if kind == "qa":
                        cp(QA[qi], gp)
                    elif kind == "ka":
                        cp(KA[c], gp)
                    else:
                        cp(T2[c][64:128, :], gp[64:128, :])
                        if qi is not None:
                            cp(QB2[qi][64:128, :], gp[0:64, :])
                    yield

                # V in natural [key, dim] layout: xsT tiles as stationary
                for rb in range(4):
                    pvn = ps_a.tile([128, 192], F32, tag="mma", name=f"pvn{c}_{rb}")
                    for d in range(6):
                        nc.tensor.matmul(
                            pvn, xsT[:, rb, d, :], wv[:, d, :],
                            start=(d == 0), stop=False,
                        )
                    nc.tensor.matmul(
                        pvn,
                        nmt[0:2, rb, :],
                        corrv[0:2, :],
                        start=False, stop=True,
                    )
                    if masked:
                        with nc.allow_low_precision(reason="bf16 V"):
                            nc.vector.tensor_scalar_mul(
                                out=vv[c][:, rb, :, 0:64].rearrange("p h f -> p (h f)"),
                                in0=pvn,
                                scalar1=maskv[:, 4 * c + rb : 4 * c + rb + 1],
                            )
                    else:
                        set_cp(4)
                        cp(vv[c][:, rb, :, 0:64], pvn.rearrange("p (h f) -> p h f", f=64))
                    yield
                for h in range(3):
                    with nc.allow_low_precision(reason="bf16 mask col"):
                        nc.gpsimd.tensor_copy(
                            out=vv[c][:, :, h, 64], in_=maskv[:, 4 * c : 4 * c + 4]
                        )

            def k_slice(h, kb, col0, ncols):
                c, q4 = kb // 4, (kb % 4) * 128
                if h == 0:
                    return KA[c][0:64, q4 + col0 : q4 + col0 + ncols]
                if h == 1:
                    return KA[c][64:128, q4 + col0 : q4 + col0 + ncols]
                return T2[c][64:128, q4 + col0 : q4 + col0 + ncols]

            def q_tile(h, qi):
                if h == 0:
                    return QA[qi][0:64, :]
                if h == 1:
                    return QA[qi][64:128, :]
                return QB2[qi][64:128, :]

            # ---------- stage B: attention, emitted as windows + finish ------
            def b_pairs(h, qi, prs, po, first, stop_last):
                qsl = q_tile(h, qi)
                for i, p in enumerate(prs):
                    kb0 = 2 * p
                    pe_ = ps_b.tile([128, 1024], F32, tag="mmb", name=f"sp{h}_{qi}_{p}")
                    nc.tensor.matmul(
                        pe_[:, 0:512], k_slice(h, kb0, 0, 128), qsl,
                        start=True, stop=True,
                    )
                    nc.tensor.matmul(
                        pe_[:, 512:1024], k_slice(h, kb0 + 1, 0, 128), qsl,
                        start=True, stop=True,
                    )
                    ee = ep.tile([128, 1024], BF, tag="exp", name=f"ee{h}_{qi}_{p}")
                    with nc.allow_low_precision(reason="bf16 attention weights"):
                        nc.scalar.activation(out=ee, in_=pe_, func=AF.Exp)
                    # each pair is a closed accumulation group (start=False
                    # reopens) so interleaved windows never hold a group open
                    nc.tensor.matmul(
                        po, vv[kb0 // 4][:, kb0 % 4, h, 0:65], ee[:, 0:512],
                        start=first, stop=False, skip_group_check=True,
                    )
                    first = False
                    nc.tensor.matmul(
                        po, vv[(kb0 + 1) // 4][:, (kb0 + 1) % 4, h, 0:65],
                        ee[:, 512:1024],
                        start=False, stop=True, skip_group_check=True,
                    )
                return first

            def b_straddles(h, qi, po, first):
                # diagonal (causal-boundary) blocks: s0(512)+s1(384)+s3(128)
                # packed in ps1; s2(256) in ps2
                q0 = q0s[qi]
                qsl = q_tile(h, qi)
                kbase = q0 // 128
                ps1 = ps_b.tile([128, 1024], F32, tag="mmb", name=f"s1_{h}_{qi}")
                ps2 = ps_b.tile([128, 1024], F32, tag="mmb", name=f"s2_{h}_{qi}")
                placing = [(ps1, 0), (ps1, 512), (ps2, 0), (ps1, 896)]
                for si, (dstp, o0) in enumerate(placing):
                    r = 128 * si
                    ns = 512 - r
                    nc.tensor.matmul(
                        dstp[:, o0 : o0 + ns],
                        k_slice(h, kbase + si, 0, 128),
                        qsl[:, r:512],
                        start=True, stop=True, skip_group_check=True,
                    )
                es1 = ep.tile([128, 1024], BF, tag="exp", name=f"e1_{h}_{qi}")
                es2 = ep.tile([128, 1024], BF, tag="exp", name=f"e2_{h}_{qi}")
                with nc.allow_low_precision(reason="bf16 attention weights"):
                    nc.scalar.activation(out=es1, in_=ps1, func=AF.Exp)
                    nc.scalar.activation(out=es2[:, 0:256], in_=ps2[:, 0:256], func=AF.Exp)
                epl = [(es1, 0), (es1, 512), (es2, 0), (es1, 896)]
                for es, o0 in epl:
                    nc.gpsimd.tensor_mul(
                        out=es[:, o0 : o0 + 128], in0=es[:, o0 : o0 + 128], in1=mb
                    )
                for si, (es, o0) in enumerate(epl):
                    r = 128 * si
                    ns = 512 - r
                    kb = kbase + si
                    nc.tensor.matmul(
                        po[:, r:512],
                        vv[kb // 4][:, kb % 4, h, 0:65],
                        es[:, o0 : o0 + ns],
                        start=first, stop=True, skip_group_check=True,
                    )
                    first = False
                return first

            def b_window(h, qi, prs, first_window, straddles=False):
                name = f"pw{qi}_{h}_{'s' if straddles else prs[0]}"
                po = ps_o.tile([65, 512], F32, tag="po", name=name)
                if straddles:
                    b_straddles(h, qi, po, True)
                    yield
                else:
                    first = True
                    for i, p in enumerate(prs):
                        first = b_pairs(
                            h, qi, (p,), po, first, stop_last=(i == len(prs) - 1)
                        )
                        yield
                if first_window:
                    nc.vector.tensor_copy(out=pb[qi][h], in_=po)
                else:
                    nc.vector.tensor_tensor(
                        out=pb[qi][h], in0=pb[qi][h], in1=po, op=ALU.add
                    )

            def b_finish(h, qi, prs, has_partial, straddles_done=False):
                set_cp(4)  # B drains mostly on DVE
                po = ps_o.tile([65, 512], F32, tag="po", name=f"po{h}_{qi}")
                first = True
                if has_partial:
                    # seed po with the windowed partial up-front (off the
                    # critical tail); all matmuls then accumulate onto it
                    nc.vector.tensor_copy(out=po, in_=pb[qi][h])
                    first = False
                first = b_pairs(h, qi, prs, po, first, stop_last=True)
                if not straddles_done:
                    first = b_straddles(h, qi, po, first)
                # normalize by denominator (row 64)
                rden = sm.tile([1, 512], BF, tag="rden", name=f"rd{h}_{qi}")
                with nc.allow_low_precision(reason="recip feeds PE broadcast"):
                    nc.vector.reciprocal(out=rden, in_=po[64:65, :])
                rdp = ps_a.tile([64, 512], F32, tag="mma", name=f"rdp{h}_{qi}")
                nc.tensor.matmul(rdp, ones_row[0:1, 0:64], rden, start=True, stop=True)
                rdb = sm.tile([64, 512], F32, tag="rdb", name=f"rdb{h}_{qi}")
                nc.vector.tensor_copy(out=rdb, in_=rdp)
                dst = oq[qi][64 * h : 64 * h + 64, :] if h < 2 else oq2[qi]
                with nc.allow_low_precision(reason="bf16 attention output"):
                    nc.vector.tensor_tensor(out=dst, in0=po[0:64, :], in1=rdb, op=ALU.mult)

            # ---------- stage C: output projection for one q-tile ----------
            def stage_c(qi):
                set_cp(2)
                for rbl in range(4):
                    rb = 4 * qi + rbl
                    a_sl = oq[qi][:, rbl * 128 : (rbl + 1) * 128]
                    b_sl = oq2[qi][:, rbl * 128 : (rbl + 1) * 128]
                    py = ps_b.tile([128, 1024], F32, tag="mmb", name=f"py{rb}")
                    nc.tensor.matmul(py[:, 0:512], a_sl, wo[:, 0:512], start=True, stop=False)
                    nc.tensor.matmul(py[:, 0:512], b_sl, wo[0:64, 768:1280], start=False, stop=True)
                    nc.tensor.matmul(py[:, 512:768], a_sl, wo[:, 512:768], start=True, stop=False)
                    nc.tensor.matmul(py[:, 512:768], b_sl, wo[0:64, 1280:1536], start=False, stop=True)
                    y_sb = wk.tile([128, D], BF, tag="y_sb", name=f"y{rb}")
                    cp(y_sb, py[:, 0:768])
                    nc.sync.dma_start(out=y_out[rb * 128 : (rb + 1) * 128, :], in_=y_sb)

            # ---------- emission: chunk order + windows + finishes ----------
            # Driver: the A-chunk must be fully emitted before this position's
            # windows (their Q/K tiles may be drained inside it — the tile
            # framework only orders reader-after-writer in emission order).
            def drive(a_gen, win_gens):
                if a_gen is not None:
                    for _ in a_gen:
                        pass
                for g in win_gens:
                    for _ in g:
                        pass

            for pos, c in enumerate(sched["order"]):
                a_gen = stage_a_chunk(c) if "A" in STAGES else None
                win_gens = []
                if "B" in STAGES:
                    for qi, winlist in sched["wins"].items():
                        for wi, (wpos, prs) in enumerate(winlist):
                            if wpos == pos:
                                for h in range(3):
                                    win_gens.append(
                                        b_window(h, qi, prs, first_window=(wi == 0))
                                    )
                    for qi, spos in sched.get("swin", {}).items():
                        if spos == pos:
                            for h in range(3):
                                win_gens.append(
                                    b_window(h, qi, (), False, straddles=True)
                                )
                drive(a_gen, win_gens)
                fin_qis = [qi for qi, (fpos, _) in sched["fins"].items() if fpos == pos]
                if "B" in STAGES:
                    for h in range(3):
                        for qi in fin_qis:
                            b_finish(
                                h, qi, sched["fins"][qi][1],
                                has_partial=qi in sched["wins"],
                                straddles_done=qi in sched.get("swin", {}),
                            )
                if "C" in STAGES:
                    for qi in fin_qis:
                        stage_c(qi)

    return nc


# --------------------------------------------------------------------------
# Host-side input prep
# --------------------------------------------------------------------------
def _prep_inputs(x, ln_g, ln_b, w_qkv, w_out, mask):
    x2d = np.asarray(x, np.float32).reshape(N, D)
    ln_g = np.asarray(ln_g, np.float32)
    ln_b = np.asarray(ln_b, np.float32)
    w_qkv = np.asarray(w_qkv, np.float32)
    w_out = np.asarray(w_out, np.float32)
    maskf = np.asarray(mask).reshape(N).astype(np.float32)
    scale = DH ** -0.5

    inner = HEADS * DH
    wq, wk_, wv_ = w_qkv[:, :inner], w_qkv[:, inner : 2 * inner], w_qkv[:, 2 * inner :]
    weff_q = (ln_g[:, None] * wq) * scale
    weff_k = ln_g[:, None] * wk_
    weff_v = ln_g[:, None] * wv_
    cb_q = (ln_b @ wq) * scale
    cb_k = ln_b @ wk_
    cb_v = ln_b @ wv_

    mb = np.triu(np.ones((128, 128), np.float32)).astype(BF16NP)

    per_core = []
    for c in range(8):
        t, role = divmod(c, 2)
        spec = ROLE_SPEC[role]
        KR = spec["key_rows"]
        KB = KR // 128
        h0 = 3 * t * DH
        sl = lambda k: slice(h0 + k * DH, h0 + (k + 1) * DH)
        # Q/K col packing: [qh0|qh1|kh0|kh1|qh2|kh2] -> [128, 6, 384]
        wqk_cols = np.concatenate(
            [
                weff_q[:, sl(0)], weff_q[:, sl(1)],
                weff_k[:, sl(0)], weff_k[:, sl(1)],
                weff_q[:, sl(2)], weff_k[:, sl(2)],
            ],
            axis=1,
        )  # [768, 384]
        cb_cols = np.concatenate(
            [
                cb_q[sl(0)], cb_q[sl(1)],
                cb_k[sl(0)], cb_k[sl(1)],
                cb_q[sl(2)], cb_k[sl(2)],
            ]
        )  # [384]
        wqk = np.ascontiguousarray(
            wqk_cols.reshape(6, 128, 384).transpose(1, 0, 2)
        ).astype(BF16NP)
        t_cols = wqk_cols.sum(axis=0)  # [384]
        corrqk = np.stack([t_cols, cb_cols]).astype(BF16NP)  # [2, 384]

        hsl = slice(h0, h0 + 3 * DH)
        wvp = np.ascontiguousarray(
            weff_v[:, hsl].reshape(6, 128, HD).transpose(1, 0, 2)
        ).astype(BF16NP)
        tv_cols = weff_v[:, hsl].sum(axis=0)
        corrv = np.stack([tv_cols, cb_v[hsl]]).astype(BF16NP)  # [2, 192]

        wo_t = w_out[hsl, :]  # [192, 768]
        wo_packed = np.zeros((128, 1536), np.float32)
        wo_packed[:, :768] = wo_t[:128]
        wo_packed[:64, 768:] = wo_t[128:]
        maskv = np.ascontiguousarray(maskf[:KR].reshape(KB, 128).T)  # [128, KB]
        per_core.append(
            dict(
                x=np.ascontiguousarray(x2d[:KR]),
                wqk=wqk,
                wv=wvp,
                corrqk=corrqk,
                corrv=corrv,
                wo=wo_packed.astype(BF16NP),
                maskv=maskv,
                mb=mb,
                ones=np.ones((1, 512), BF16NP),
            )
        )
    return per_core


def _get_runners(masked=False):
    global _RUNNERS
    if _RUNNERS is None or _RUNNERS[2] != masked:
        _install_tile_patch()
        _RUNNERS = [
            _make_runner(_build_role_program(0, masked)),
            _make_runner(_build_role_program(1, masked)),
            masked,
        ]
    return _RUNNERS


def kernel(x, ln_g, ln_b, w_qkv, w_out, mask):
    import jax

    runners = _get_runners(masked=not np.asarray(mask).all())
    per_core = _prep_inputs(x, ln_g, ln_b, w_qkv, w_out, mask)
    devs = jax.devices()
    futs = [
        runners[c % 2](per_core[c], devs[c], core_id=c) for c in range(8)
    ]
    outs = [np.asarray(f["out"]).astype(np.float32) for f in futs]

    full = np.zeros((N, D), np.float32)
    for t in range(4):
        for role in (0, 1):
            o = outs[2 * t + role]
            for qi, q0 in enumerate(ROLE_SPEC[role]["q0s"]):
                full[q0 : q0 + 512] += o[qi * 512 : (qi + 1) * 512]
    return full.reshape(np.asarray(x).shape).astype(np.float32)


# revision 52
# speedup vs baseline: 1.3091x; 1.3091x over previous
"""Trainium2 Bass kernel for nn_BaseSelfAttention_88433376625006.

Computes: LayerNorm -> QKV projection -> 12-head causal self-attention
(seq 4096, dim 768) -> output projection, on 8 NeuronCores.

Sharding: 4 teams x 2 cores. Team t owns heads {3t, 3t+1, 3t+2}. Within a
team, core role 0 handles query rows {0..1023, 3072..4095} and role 1 rows
{1024..3071} (equal causal work). Each core computes LN + K/V for the keys
it needs, flash-style attention with the sim matrix in [k, q] layout, and a
partial output projection over its heads; the host scatters rows and sums
the 4 team partials. No collectives.

v2 design notes:
- All matmul inputs are bf16 (full PE rate, half SBUF, 2e-2 tolerance is
  plenty). PSUM stays f32; drains convert to bf16.
- No PE transposes: x rows are scaled by rstd (xs = x*rstd, bf16) and
  transposed by the DMA XBAR (SBUF->SBUF, 16x128-tile ucode) into [D, seq]
  layout. The remaining LayerNorm terms (nm_n*colsum(W) + ln_b@W, with
  nm = -mu*rstd) are added by one rank-2 correction matmul per group:
  a [128, 512] stats tile holding nm at cols {0,128,256,384} and ones
  elsewhere is DMA-transposed so partitions {0,1} carry the full-width
  [nm; ones] rows, matching a host-packed [t_cols; cb_cols] operand.
- rstd = exp(-0.5*ln(var+eps)) keeps ACT inside the exp/ln/copy activation
  table: a Sqrt would force a 1.3us table reload next to every attention
  exp batch.
- Streaming schedule (SCHED): chunks processed q-chunks-early; attention
  pairs for late q-tiles are emitted in windows as key chunks land,
  accumulated in PSUM and flushed to SBUF partials, so the PE never idles
  through the key-only middle chunks. Finishes seed PSUM with the partial,
  add leftover pairs + causal-boundary straddles, then normalize.
- Softmax skips max-subtraction (sim is O(1)); denominator rides the
  attention matmul as a ones-column of V.
- Output is written bf16 and upcast on host.
"""

import numpy as np
import ml_dtypes

BF16NP = ml_dtypes.bfloat16

HEADS = 12
N = 4096
D = 768
DH = 64
LN_EPS = 1e-5
TEAM_HEADS = 3
HD = TEAM_HEADS * DH  # head dims per core = 192

ROLE_SPEC = {
    0: dict(key_rows=4096, q0s=(0, 512, 3072, 3584)),
    1: dict(key_rows=3072, q0s=(1024, 1536, 2048, 2560)),
}

# Streaming schedule: chunks are processed in `order` (q-chunks early so Q is
# available); attention pairs for deferred q-tiles are emitted in windows as
# their key chunks complete, accumulating into SBUF partials; `fins` holds the
# straddle + leftover pairs + merge + normalize.  Entries: wins[qi] =
# [(position, pairs)...], fins[qi] = (position, leftover_pairs).
SCHED = {
    0: dict(
        order=[0, 1, 6, 5, 7, 2, 3, 4],
        wins={2: [(2, (0, 1, 2, 3)), (3, (10, 11)), (5, (4, 5)), (6, (6, 7))],
              3: [(4, (0, 1, 2, 3, 12, 13)), (5, (4, 5, 10, 11)), (6, (6, 7))]},
        swin={},
        fins={0: (0, ()), 1: (1, (0, 1)), 2: (7, (8, 9)), 3: (7, (8, 9))},
    ),
    1: dict(
        order=[0, 2, 3, 4, 5, 1],
        wins={0: [(1, (0, 1))], 1: [(2, (0, 1, 4, 5))],
              2: [(3, (0, 1, 4, 5, 6, 7))], 3: [(4, (0, 1, 4, 5, 6, 7, 8, 9))]},
        swin={},
        fins={0: (5, (2, 3)), 1: (5, (2, 3)), 2: (5, (2, 3)), 3: (5, (2, 3))},
    ),
}

_RUNNERS = None  # lazy build cache
STAGES = "ABC"  # debug: which stages to emit
XS_ON_ACT = False  # xs = x*rstd on ACT (Identity+scale) instead of DVE


# --------------------------------------------------------------------------
# neuronxcc workaround: this build rejects instructions with >1 sync wait.
# --------------------------------------------------------------------------
def _install_tile_patch():
    import concourse.tile as tile
    from concourse import mybir
    from concourse.vector_clock import ScopedClock

    if getattr(tile.TileContext, "_single_wait_patch", False):
        return

    def _patched_drain_and_barrier(self, tick_clock, wait_clock):
        nc = self.nc
        probe = nc.sync.nop(nofuse=True, hint="split_drain_waits")
        wait_clock.add_sem_waits(
            probe.ins, ScopedClock({None: tick_clock.global_clock})
        )
        si = probe.ins.sync_info
        waits = list(si.on_wait) if si and si.on_wait else []
        if len(waits) > 1:
            si.on_wait = waits[:1]
            for i in range(1, len(waits)):
                extra = nc.sync.nop(nofuse=True, hint=f"split_drain_waits_{i}")
                xsi = extra.ins.sync_info
                if xsi is None:
                    extra.ins.sync_info = mybir.SyncInfo(
                        on_wait=[waits[i]], on_update=[]
                    )
                else:
                    xsi.on_wait = [waits[i]]
        nc.sync.drain()
        nc.all_engine_barrier()
        popped = nc._tile_sem_poison_stack.pop()
        assert popped is self._sem_poison
        nc.clear_and_free_semaphores(list(self.sems.allocated().values()))
        nc.all_engine_barrier()

    tile.TileContext._drain_and_barrier = _patched_drain_and_barrier

    _orig_commit = tile.TileContext._commit_instruction

    def _patched_commit_instruction(self, inst, lazy_reg_writes=True):
        si = getattr(inst, "sync_info", None)
        if (
            si is not None
            and si.on_wait
            and len(si.on_wait) > 1
            and inst.engine != mybir.EngineType.Unassigned
        ):
            waits = list(si.on_wait)
            si.on_wait = waits[-1:]
            for w in waits[:-1]:
                nop = mybir.InstNoOp(
                    name=self.nc.get_next_instruction_name(),
                    sync_info=mybir.SyncInfo(on_wait=[w], on_update=[]),
                    bass_nofuse=True,
                    engine=inst.engine,
                )
                _orig_commit(self, nop, lazy_reg_writes=False)
        return _orig_commit(self, inst, lazy_reg_writes=lazy_reg_writes)

    tile.TileContext._commit_instruction = _patched_commit_instruction
    tile.TileContext._single_wait_patch = True


# --------------------------------------------------------------------------
# Per-device program dispatch (different programs on different cores).
# --------------------------------------------------------------------------
def _make_runner(nc):
    import jax
    from concourse import mybir
    from concourse.bass2jax import _bass_exec_p, install_neuronx_cc_hook

    install_neuronx_cc_hook()
    pid_name = nc.partition_id_tensor.name if nc.partition_id_tensor else None
    in_names, out_names, out_avals, zero_outs = [], [], [], []
    for alloc in nc.m.functions[0].allocations:
        if not isinstance(alloc, mybir.MemoryLocationSet):
            continue
        name = alloc.memorylocations[0].name
        if alloc.kind == "ExternalInput":
            if name != pid_name:
                in_names.append(name)
        elif alloc.kind == "ExternalOutput":
            shape = tuple(alloc.tensor_shape)
            dtype = mybir.dt.np(alloc.dtype)
            out_names.append(name)
            out_avals.append(jax.core.ShapedArray(shape, dtype))
            zero_outs.append(np.zeros(shape, dtype))
    n_params = len(in_names)
    all_names = in_names + out_names + ([pid_name] if pid_name else [])
    donate = tuple(range(n_params, n_params + len(out_names)))

    def _body(*args):
        return tuple(
            _bass_exec_p.bind(
                *args,
                out_avals=tuple(out_avals),
                in_names=tuple(all_names),
                out_names=tuple(out_names),
                lowering_input_output_aliases=(),
                sim_require_finite=True,
                sim_require_nnan=True,
                nc=nc,
            )
        )

    jitted = jax.jit(_body, donate_argnums=donate, keep_unused=True)
    jitted_nodonate = jax.jit(_body, keep_unused=True)

    def run(in_map, device, core_id=0):
        args = [jax.device_put(np.asarray(in_map[n]), device) for n in in_names]
        args += [jax.device_put(z.copy(), device) for z in zero_outs]
        if pid_name is not None:
            args.append(jax.device_put(np.array([[core_id]], np.uint32), device))
        outs = jitted(*args)
        return {n: outs[i] for i, n in enumerate(out_names)}

    def stage(in_map, device, core_id=0):
        args = [jax.device_put(np.asarray(in_map[n]), device) for n in in_names]
        args += [jax.device_put(z, device) for z in zero_outs]
        if pid_name is not None:
            args.append(jax.device_put(np.array([[core_id]], np.uint32), device))
        return args

    def run_staged(args):
        return jitted_nodonate(*args)

    run.stage = stage
    run.run_staged = run_staged
    run.out_names = out_names
    return run


# --------------------------------------------------------------------------
# The kernel program for one role.
# --------------------------------------------------------------------------
def _build_role_program(role, masked=False):
    import concourse.bass as bass
    import concourse.tile as tile
    from concourse import mybir

    F32 = mybir.dt.float32
    BF = mybir.dt.bfloat16
    AF = mybir.ActivationFunctionType
    ALU = mybir.AluOpType

    spec = ROLE_SPEC[role]
    KR = spec["key_rows"]  # key rows this core needs
    q0s = spec["q0s"]  # global start row of each 512-row query tile
    KC = KR // 512  # number of 512-row chunks
    KB = KR // 128  # number of 128-row key blocks
    q_chunks = {q0 // 512: qi for qi, q0 in enumerate(q0s)}  # chunk -> q index
    sched = SCHED[role]

    nc = bass.Bass(enable_partition_id=False)

    x_in = nc.declare_dram_parameter("x", [KR, D], F32, isOutput=False)
    wqk_in = nc.declare_dram_parameter("wqk", [128, 6, 384], BF, isOutput=False)
    wv_in = nc.declare_dram_parameter("wv", [128, 6, 192], BF, isOutput=False)
    cqk_in = nc.declare_dram_parameter("corrqk", [2, 384], BF, isOutput=False)
    cv_in = nc.declare_dram_parameter("corrv", [2, 192], BF, isOutput=False)
    wo_in = nc.declare_dram_parameter("wo", [128, 1536], BF, isOutput=False)
    mk_in = nc.declare_dram_parameter("maskv", [128, KB], F32, isOutput=False)
    mb_in = nc.declare_dram_parameter("mb", [128, 128], BF, isOutput=False)
    on_in = nc.declare_dram_parameter("ones", [1, 512], BF, isOutput=False)
    y_out = nc.declare_dram_parameter("out", [2048, D], BF, isOutput=True)

    with tile.TileContext(nc) as tc:
        with (
            tc.tile_pool(name="persist", bufs=1) as pp,
            tc.tile_pool(name="work", bufs=3) as wk,
            tc.tile_pool(name="xstp", bufs=4) as xsp,
            tc.tile_pool(name="xntp", bufs=3) as xp,
            tc.tile_pool(name="xtp", bufs=6) as xtp,
            tc.tile_pool(name="small", bufs=6) as sm,
            tc.tile_pool(name="expp", bufs=6) as ep,
            tc.tile_pool(name="psga", bufs=2, space="PSUM") as ps_a,
            tc.tile_pool(name="psim", bufs=2, space="PSUM") as ps_b,
            tc.tile_pool(name="pso", bufs=1, space="PSUM") as ps_o,
        ):
            # ---- persistent tiles ----
            ones_row = pp.tile([1, 512], BF, tag="ones_row")
            nc.sync.dma_start(out=ones_row, in_=on_in[:])
            maskv = pp.tile([128, KB], F32, tag="maskv")
            nc.sync.dma_start(out=maskv, in_=mk_in[:])
            mb = pp.tile([128, 128], BF, tag="mb")
            nc.sync.dma_start(out=mb, in_=mb_in[:])
            eps_t = pp.tile([128, 1], F32, tag="eps")
            nc.vector.memset(eps_t, LN_EPS)
            wqk = pp.tile([128, 6, 384], BF, tag="wqk")
            nc.gpsimd.dma_start(out=wqk, in_=wqk_in[:])
            wv = pp.tile([128, 6, 192], BF, tag="wv")
            nc.gpsimd.dma_start(out=wv, in_=wv_in[:])
            corrqk = pp.tile([2, 384], BF, tag="corrqk")
            nc.gpsimd.dma_start(out=corrqk, in_=cqk_in[:])
            corrv = pp.tile([2, 192], BF, tag="corrv")
            nc.gpsimd.dma_start(out=corrv, in_=cv_in[:])
            wo = pp.tile([128, 1536], BF, tag="wo")
            nc.gpsimd.dma_start(out=wo, in_=wo_in[:])

            # per-chunk / per-qtile persistent tiles => fine-grained deps
            # QA[qi]: qh0 on p0-63, qh1 on p64-127.  QB2[qi]: qh2 on p64-127.
            # KA[c]: kh0 p0-63, kh1 p64-127.  T2[c]: kh2 on p64-127.
            QA = [pp.tile([128, 512], BF, name=f"QA{qi}", tag=f"QA{qi}") for qi in range(4)]
            QB2 = [pp.tile([128, 512], BF, name=f"QB2_{qi}", tag=f"QB2_{qi}") for qi in range(4)]
            KA = [pp.tile([128, 512], BF, name=f"KA{c}", tag=f"KA{c}") for c in range(KC)]
            T2 = [pp.tile([128, 512], BF, name=f"T2_{c}", tag=f"T2_{c}") for c in range(KC)]
            vv = [
                pp.tile([128, 4, 3, 66], BF, name=f"vv{c}", tag=f"vv{c}")
                for c in range(KC)
            ]
            oq = [pp.tile([128, 512], BF, name=f"oq{qi}", tag=f"oq{qi}") for qi in range(4)]
            oq2 = [pp.tile([64, 512], BF, name=f"oq2_{qi}", tag=f"oq2_{qi}") for qi in range(4)]
            # SBUF partial accumulators for windowed attention (f32)
            pb = {
                qi: [
                    pp.tile([65, 512], F32, name=f"pb{qi}_{h}", tag=f"pb{qi}_{h}")
                    for h in range(3)
                ]
                for qi in sched["wins"]
            }

            # psum->sbuf drains, round-robin with a per-stage ACT share.
            # set_cp(k>0): 1/k of copies on ACT; set_cp(k<0): 1/|k| on DVE.
            _cp_state = [0, 2]

            def cp(out, in_):
                _cp_state[0] += 1
                k = _cp_state[1]
                on_act = (
                    _cp_state[0] % k == 0 if k > 0 else _cp_state[0] % (-k) != 0
                )
                with nc.allow_low_precision(reason="psum drain rounds to bf16"):
                    if on_act:
                        nc.scalar.copy(out=out, in_=in_)
                    else:
                        nc.vector.tensor_copy(out=out, in_=in_)

            def set_cp(act_every):
                _cp_state[1] = act_every

            # ---------- stage A: LN + DMA-transpose + QKV for one chunk ----
            # Emitted as a generator of PE-sized steps so the emission driver
            # can interleave attention pairs (ACT-heavy) with QKV groups
            # (PE-heavy) in the in-order engine queues.
            def stage_a_chunk(c):
                set_cp(4)  # stage-A drains mostly on DVE (ACT busy with exp)
                qi = q_chunks.get(c)
                mvs = sm.tile([128, 4, 2], F32, tag="mvs", name=f"mvs{c}")
                x_ts = []
                for rb in range(4):
                    row0 = c * 512 + rb * 128
                    x_t = xtp.tile([128, D], F32, tag="x_t", name=f"x{c}_{rb}")
                    x_ts.append(x_t)
                    nc.sync.dma_start(out=x_t, in_=x_in[row0 : row0 + 128, :])
                    st = sm.tile([128, 2, 6], F32, tag="st", name=f"st{c}_{rb}")
                    nc.vector.bn_stats(out=st[:, 0, :], in_=x_t[:, 0:512])
                    nc.vector.bn_stats(out=st[:, 1, :], in_=x_t[:, 512:768])
                    nc.vector.bn_aggr(out=mvs[:, rb, :], in_=st)
                # rstd = exp(-0.5*ln(var+eps)): keeps ACT inside the exp/ln/copy
                # activation table (Sqrt would force a 1.3us table reload per use)
                lns = sm.tile([128, 4], F32, tag="lns", name=f"lns{c}")
                rstds = sm.tile([128, 4], F32, tag="rstds", name=f"rss{c}")
                if c == 0:  # latency-critical first chunk: per-rowblock chain
                    for rb in range(4):
                        nc.scalar.activation(
                            out=lns[:, rb : rb + 1], in_=mvs[:, rb, 1:2],
                            func=AF.Ln, bias=eps_t, scale=1.0,
                        )
                        nc.scalar.activation(
                            out=rstds[:, rb : rb + 1], in_=lns[:, rb : rb + 1],
                            func=AF.Exp, bias=0.0, scale=-0.5,
                        )
                else:
                    nc.scalar.activation(
                        out=lns, in_=mvs[:, :, 1], func=AF.Ln, bias=eps_t, scale=1.0
                    )
                    nc.scalar.activation(
                        out=rstds, in_=lns, func=AF.Exp, bias=0.0, scale=-0.5
                    )

                # nm tile: col 128*rb = -mu*rstd of rowblock rb, other cols = 1.
                # DMA-transposed into nmt [128, 4, 128]: partition 0 then holds
                # the full-width row nm_n over queries n=(rb*128+j); partition 1
                # holds ones.
                nmb = sm.tile([128, 512], BF, tag="nmb", name=f"nmb{c}")
                nc.gpsimd.memset(nmb, 1.0)
                negm = sm.tile([128, 4], F32, tag="negm", name=f"negm{c}")
                nc.gpsimd.tensor_scalar(
                    out=negm, in0=mvs[:, :, 0], scalar1=-1.0, scalar2=None,
                    op0=ALU.mult,
                )
                with nc.allow_low_precision(reason="LN correction rounds to bf16"):
                    nc.gpsimd.tensor_tensor(
                        out=nmb[:, 0:512:128], in0=negm, in1=rstds, op=ALU.mult
                    )
                nmt = sm.tile([128, 4, 128], BF, tag="nmt", name=f"nmt{c}")
                nc.sync.dma_start_transpose(out=nmt, in_=nmb)

                # xs = x*rstd (bf16), DMA-transposed into [D, seq] layout
                xsT = xp.tile([128, 4, 6, 128], BF, tag="xsT", name=f"xsT{c}")
                for rb in range(4):
                    xs = xsp.tile([128, D], BF, tag="xs", name=f"xs{c}_{rb}")
                    with nc.allow_low_precision(reason="xs rounds to bf16"):
                        if XS_ON_ACT:
                            nc.scalar.activation(
                                out=xs, in_=x_ts[rb], func=AF.Identity,
                                bias=0.0, scale=rstds[:, rb : rb + 1],
                            )
                        else:
                            # on Pool: DVE is loaded with bn_stats/drains, Pool idles
                            nc.gpsimd.tensor_scalar(
                                out=xs, in0=x_ts[rb], scalar1=rstds[:, rb : rb + 1],
                                scalar2=None, op0=ALU.mult,
                            )
                    nc.sync.dma_start_transpose(out=xsT[:, rb, :, :], in_=xs)
                yield  # front matter emitted; PE steps follow

                # Q/K groups: cols 0-127=[qh0|qh1], 128-255=[kh0|kh1],
                # 256-383=[qh2|kh2]
                groups = ([(0, "qa")] if qi is not None else []) + [
                    (128, "ka"),
                    (256, "t2"),
                ]
                for g0, kind in groups:
                    gp = ps_a.tile([128, 512], F32, tag="mma", name=f"gp{c}_{g0}")
                    for d in range(6):
                        nc.tensor.matmul(
                            gp, wqk[:, d, g0 : g0 + 128], xsT[:, :, d, :],
                            start=(d == 0), stop=False,
                        )
                    nc.tensor.matmul(
                        gp, corrqk[0:2, g0 : g0 + 128], nmt[0:2, :, :],
                        start=False, stop=True,
                    )
                    set_cp(4)
                    if kind == "qa":
                        cp(QA[qi], gp)
                    elif kind == "ka":
                        cp(KA[c], gp)
                    else:
                        cp(T2[c][64:128, :], gp[64:128, :])
                        if qi is not None:
                            cp(QB2[qi][64:128, :], gp[0:64, :])
                    yield

                # V in natural [key, dim] layout: xsT tiles as stationary
                for rb in range(4):
                    pvn = ps_a.tile([128, 192], F32, tag="mma", name=f"pvn{c}_{rb}")
                    for d in range(6):
                        nc.tensor.matmul(
                            pvn, xsT[:, rb, d, :], wv[:, d, :],
                            start=(d == 0), stop=False,
                        )
                    nc.tensor.matmul(
                        pvn,
                        nmt[0:2, rb, :],
                        corrv[0:2, :],
                        start=False, stop=True,
                    )
                    if masked:
                        with nc.allow_low_precision(reason="bf16 V"):
                            nc.vector.tensor_scalar_mul(
                                out=vv[c][:, rb, :, 0:64].rearrange("p h f -> p (h f)"),
                                in0=pvn,
                                scalar1=maskv[:, 4 * c + rb : 4 * c + rb + 1],
                            )
                    else:
                        set_cp(4)
                        cp(vv[c][:, rb, :, 0:64], pvn.rearrange("p (h f) -> p h f", f=64))
                    yield
                for h in range(3):
                    with nc.allow_low_precision(reason="bf16 mask col"):
                        nc.gpsimd.tensor_copy(
                            out=vv[c][:, :, h, 64], in_=maskv[:, 4 * c : 4 * c + 4]
                        )

            def k_slice(h, kb, col0, ncols):
                c, q4 = kb // 4, (kb % 4) * 128
                if h == 0:
                    return KA[c][0:64, q4 + col0 : q4 + col0 + ncols]
                if h == 1:
                    return KA[c][64:128, q4 + col0 : q4 + col0 + ncols]
                return T2[c][64:128, q4 + col0 : q4 + col0 + ncols]

            def q_tile(h, qi):
                if h == 0:
                    return QA[qi][0:64, :]
                if h == 1:
                    return QA[qi][64:128, :]
                return QB2[qi][64:128, :]

            # ---------- stage B: attention, emitted as windows + finish ------
            def b_pairs(h, qi, prs, po, first, stop_last):
                qsl = q_tile(h, qi)
                for i, p in enumerate(prs):
                    kb0 = 2 * p
                    pe_ = ps_b.tile([128, 1024], F32, tag="mmb", name=f"sp{h}_{qi}_{p}")
                    nc.tensor.matmul(
                        pe_[:, 0:512], k_slice(h, kb0, 0, 128), qsl,
                        start=True, stop=True,
                    )
                    nc.tensor.matmul(
                        pe_[:, 512:1024], k_slice(h, kb0 + 1, 0, 128), qsl,
                        start=True, stop=True,
                    )
                    ee = ep.tile([128, 1024], BF, tag="exp", name=f"ee{h}_{qi}_{p}")
                    with nc.allow_low_precision(reason="bf16 attention weights"):
                        nc.scalar.activation(out=ee, in_=pe_, func=AF.Exp)
                    # each pair is a closed accumulation group (start=False
                    # reopens) so interleaved windows never hold a group open
                    nc.tensor.matmul(
                        po, vv[kb0 // 4][:, kb0 % 4, h, 0:65], ee[:, 0:512],
                        start=first, stop=False, skip_group_check=True,
                    )
                    first = False
                    nc.tensor.matmul(
                        po, vv[(kb0 + 1) // 4][:, (kb0 + 1) % 4, h, 0:65],
                        ee[:, 512:1024],
                        start=False, stop=True, skip_group_check=True,
                    )
                return first

            def b_straddles(h, qi, po, first):
                # diagonal (causal-boundary) blocks: s0(512)+s1(384)+s3(128)
                # packed in ps1; s2(256) in ps2
                q0 = q0s[qi]
                qsl = q_tile(h, qi)
                kbase = q0 // 128
                ps1 = ps_b.tile([128, 1024], F32, tag="mmb", name=f"s1_{h}_{qi}")
                ps2 = ps_b.tile([128, 1024], F32, tag="mmb", name=f"s2_{h}_{qi}")
                placing = [(ps1, 0), (ps1, 512), (ps2, 0), (ps1, 896)]
                for si, (dstp, o0) in enumerate(placing):
                    r = 128 * si
                    ns = 512 - r
                    nc.tensor.matmul(
                        dstp[:, o0 : o0 + ns],
                        k_slice(h, kbase + si, 0, 128),
                        qsl[:, r:512],
                        start=True, stop=True, skip_group_check=True,
                    )
                es1 = ep.tile([128, 1024], BF, tag="exp", name=f"e1_{h}_{qi}")
                es2 = ep.tile([128, 1024], BF, tag="exp", name=f"e2_{h}_{qi}")
                with nc.allow_low_precision(reason="bf16 attention weights"):
                    nc.scalar.activation(out=es1, in_=ps1, func=AF.Exp)
                    nc.scalar.activation(out=es2[:, 0:256], in_=ps2[:, 0:256], func=AF.Exp)
                epl = [(es1, 0), (es1, 512), (es2, 0), (es1, 896)]
                for es, o0 in epl:
                    nc.gpsimd.tensor_mul(
                        out=es[:, o0 : o0 + 128], in0=es[:, o0 : o0 + 128], in1=mb
                    )
                for si, (es, o0) in enumerate(epl):
                    r = 128 * si
                    ns = 512 - r
                    kb = kbase + si
                    nc.tensor.matmul(
                        po[:, r:512],
                        vv[kb // 4][:, kb % 4, h, 0:65],
                        es[:, o0 : o0 + ns],
                        start=first, stop=True, skip_group_check=True,
                    )
                    first = False
                return first

            def b_window(h, qi, prs, first_window, straddles=False):
                name = f"pw{qi}_{h}_{'s' if straddles else prs[0]}"
                po = ps_o.tile([65, 512], F32, tag="po", name=name)
                if straddles:
                    b_straddles(h, qi, po, True)
                    yield
                else:
                    first = True
                    for i, p in enumerate(prs):
                        first = b_pairs(
                            h, qi, (p,), po, first, stop_last=(i == len(prs) - 1)
                        )
                        yield
                if first_window:
                    nc.vector.tensor_copy(out=pb[qi][h], in_=po)
                else:
                    nc.vector.tensor_tensor(
                        out=pb[qi][h], in0=pb[qi][h], in1=po, op=ALU.add
                    )

            def b_finish(h, qi, prs, has_partial, straddles_done=False):
                set_cp(4)  # B drains mostly on DVE
                po = ps_o.tile([65, 512], F32, tag="po", name=f"po{h}_{qi}")
                first = True
                if has_partial:
                    # seed po with the windowed partial up-front (off the
                    # critical tail); all matmuls then accumulate onto it
                    nc.vector.tensor_copy(out=po, in_=pb[qi][h])
                    first = False
                first = b_pairs(h, qi, prs, po, first, stop_last=True)
                if not straddles_done:
                    first = b_straddles(h, qi, po, first)
                # normalize by denominator (row 64)
                rden = sm.tile([1, 512], BF, tag="rden", name=f"rd{h}_{qi}")
                with nc.allow_low_precision(reason="recip feeds PE broadcast"):
                    nc.vector.reciprocal(out=rden, in_=po[64:65, :])
                rdp = ps_a.tile([64, 512], F32, tag="mma", name=f"rdp{h}_{qi}")
                nc.tensor.matmul(rdp, ones_row[0:1, 0:64], rden, start=True, stop=True)
                rdb = sm.tile([64, 512], F32, tag="rdb", name=f"rdb{h}_{qi}")
                nc.vector.tensor_copy(out=rdb, in_=rdp)
                dst = oq[qi][64 * h : 64 * h + 64, :] if h < 2 else oq2[qi]
                with nc.allow_low_precision(reason="bf16 attention output"):
                    nc.vector.tensor_tensor(out=dst, in0=po[0:64, :], in1=rdb, op=ALU.mult)

            # ---------- stage C: output projection for one q-tile ----------
            def stage_c(qi):
                set_cp(2)
                for rbl in range(4):
                    rb = 4 * qi + rbl
                    a_sl = oq[qi][:, rbl * 128 : (rbl + 1) * 128]
                    b_sl = oq2[qi][:, rbl * 128 : (rbl + 1) * 128]
                    py = ps_b.tile([128, 1024], F32, tag="mmb", name=f"py{rb}")
                    nc.tensor.matmul(py[:, 0:512], a_sl, wo[:, 0:512], start=True, stop=False)
                    nc.tensor.matmul(py[:, 0:512], b_sl, wo[0:64, 768:1280], start=False, stop=True)
                    nc.tensor.matmul(py[:, 512:768], a_sl, wo[:, 512:768], start=True, stop=False)
                    nc.tensor.matmul(py[:, 512:768], b_sl, wo[0:64, 1280:1536], start=False, stop=True)
                    y_sb = wk.tile([128, D], BF, tag="y_sb", name=f"y{rb}")
                    cp(y_sb, py[:, 0:768])
                    nc.sync.dma_start(out=y_out[rb * 128 : (rb + 1) * 128, :], in_=y_sb)

            # ---------- emission: chunk order + windows + finishes ----------
            # Driver: the A-chunk must be fully emitted before this position's
            # windows (their Q/K tiles may be drained inside it — the tile
            # framework only orders reader-after-writer in emission order).
            def drive(a_gen, win_gens):
                if a_gen is not None:
                    for _ in a_gen:
                        pass
                for g in win_gens:
                    for _ in g:
                        pass

            for pos, c in enumerate(sched["order"]):
                a_gen = stage_a_chunk(c) if "A" in STAGES else None
                win_gens = []
                if "B" in STAGES:
                    for qi, winlist in sched["wins"].items():
                        for wi, (wpos, prs) in enumerate(winlist):
                            if wpos == pos:
                                for h in range(3):
                                    win_gens.append(
                                        b_window(h, qi, prs, first_window=(wi == 0))
                                    )
                    for qi, spos in sched.get("swin", {}).items():
                        if spos == pos:
                            for h in range(3):
                                win_gens.append(
                                    b_window(h, qi, (), False, straddles=True)
                                )
                drive(a_gen, win_gens)
                fin_qis = [qi for qi, (fpos, _) in sched["fins"].items() if fpos == pos]
                if "B" in STAGES:
                    for h in range(3):
                        for qi in fin_qis:
                            b_finish(
                                h, qi, sched["fins"][qi][1],
                                has_partial=qi in sched["wins"],
                                straddles_done=qi in sched.get("swin", {}),
                            )
                if "C" in STAGES:
                    for qi in fin_qis:
                        stage_c(qi)

    return nc


# --------------------------------------------------------------------------
# Host-side input prep
# --------------------------------------------------------------------------
def _prep_inputs(x, ln_g, ln_b, w_qkv, w_out, mask):
    x2d = np.asarray(x, np.float32).reshape(N, D)
    ln_g = np.asarray(ln_g, np.float32)
    ln_b = np.asarray(ln_b, np.float32)
    w_qkv = np.asarray(w_qkv, np.float32)
    w_out = np.asarray(w_out, np.float32)
    maskf = np.asarray(mask).reshape(N).astype(np.float32)
    scale = DH ** -0.5

    inner = HEADS * DH
    wq, wk_, wv_ = w_qkv[:, :inner], w_qkv[:, inner : 2 * inner], w_qkv[:, 2 * inner :]
    weff_q = (ln_g[:, None] * wq) * scale
    weff_k = ln_g[:, None] * wk_
    weff_v = ln_g[:, None] * wv_
    cb_q = (ln_b @ wq) * scale
    cb_k = ln_b @ wk_
    cb_v = ln_b @ wv_

    mb = np.triu(np.ones((128, 128), np.float32)).astype(BF16NP)

    per_core = []
    for c in range(8):
        t, role = divmod(c, 2)
        spec = ROLE_SPEC[role]
        KR = spec["key_rows"]
        KB = KR // 128
        h0 = 3 * t * DH
        sl = lambda k: slice(h0 + k * DH, h0 + (k + 1) * DH)
        # Q/K col packing: [qh0|qh1|kh0|kh1|qh2|kh2] -> [128, 6, 384]
        wqk_cols = np.concatenate(
            [
                weff_q[:, sl(0)], weff_q[:, sl(1)],
                weff_k[:, sl(0)], weff_k[:, sl(1)],
                weff_q[:, sl(2)], weff_k[:, sl(2)],
            ],
            axis=1,
        )  # [768, 384]
        cb_cols = np.concatenate(
            [
                cb_q[sl(0)], cb_q[sl(1)],
                cb_k[sl(0)], cb_k[sl(1)],
                cb_q[sl(2)], cb_k[sl(2)],
            ]
        )  # [384]
        wqk = np.ascontiguousarray(
            wqk_cols.reshape(6, 128, 384).transpose(1, 0, 2)
        ).astype(BF16NP)
        t_cols = wqk_cols.sum(axis=0)  # [384]
        corrqk = np.stack([t_cols, cb_cols]).astype(BF16NP)  # [2, 384]

        hsl = slice(h0, h0 + 3 * DH)
        wvp = np.ascontiguousarray(
            weff_v[:, hsl].reshape(6, 128, HD).transpose(1, 0, 2)
        ).astype(BF16NP)
        tv_cols = weff_v[:, hsl].sum(axis=0)
        corrv = np.stack([tv_cols, cb_v[hsl]]).astype(BF16NP)  # [2, 192]

        wo_t = w_out[hsl, :]  # [192, 768]
        wo_packed = np.zeros((128, 1536), np.float32)
        wo_packed[:, :768] = wo_t[:128]
        wo_packed[:64, 768:] = wo_t[128:]
        maskv = np.ascontiguousarray(maskf[:KR].reshape(KB, 128).T)  # [128, KB]
        per_core.append(
            dict(
                x=np.ascontiguousarray(x2d[:KR]),
                wqk=wqk,
                wv=wvp,
                corrqk=corrqk,
                corrv=corrv,
                wo=wo_packed.astype(BF16NP),
                maskv=maskv,
                mb=mb,
                ones=np.ones((1, 512), BF16NP),
            )
        )
    return per_core


def _get_runners(masked=False):
    global _RUNNERS
    if _RUNNERS is None or _RUNNERS[2] != masked:
        _install_tile_patch()
        _RUNNERS = [
            _make_runner(_build_role_program(0, masked)),
            _make_runner(_build_role_program(1, masked)),
            masked,
        ]
    return _RUNNERS


def kernel(x, ln_g, ln_b, w_qkv, w_out, mask):
    import jax

    runners = _get_runners(masked=not np.asarray(mask).all())
    per_core = _prep_inputs(x, ln_g, ln_b, w_qkv, w_out, mask)
    devs = jax.devices()
    futs = [
        runners[c % 2](per_core[c], devs[c], core_id=c) for c in range(8)
    ]
    outs = [np.asarray(f["out"]).astype(np.float32) for f in futs]

    full = np.zeros((N, D), np.float32)
    for t in range(4):
        for role in (0, 1):
            o = outs[2 * t + role]
            for qi, q0 in enumerate(ROLE_SPEC[role]["q0s"]):
                full[q0 : q0 + 512] += o[qi * 512 : (qi + 1) * 512]
    return full.reshape(np.asarray(x).shape).astype(np.float32)
